# revision 1
# baseline (speedup 1.0000x reference)
"""Trainium2 Bass kernel: visibility prediction (softplus -> 3x3 Hann conv -> type-2 NuDFT).

vis[k] = cell^2 * sum_{y,x} conv(softplus(base_cube))[y,x]
         * exp(-2i*pi*u_k*c_x) * exp(-2i*pi*v_k*c_y)

Separable NuDFT, sharded over the nvis dimension across 8 NeuronCores.
Per chunk of 128 visibilities (vis index on partitions):
  - u-side phase matrices in (x, k) layout: PE outer products give
    [-q | q+0.25] (negated-coords weights + 0.25-bias matmul), range
    reduction r = x - round(x) via the magic-number trick
    (round(x) = (x + 1.5*2^23) - 1.5*2^23, exact in f32), one ACT Sin
    pass over both halves -> [a_im | a_re] = [-sin(2*pi*q) | cos(2*pi*q)].
  - T matmuls on PE accumulate [T_im | T_re | T_imneg] in PSUM.
  - v-side phases via per-partition tensor_scalar + same rounding
    (on GPSIMD) + ACT Sin -> [Cv | Svn].
  - combine + reduce fused into 2 scalar_tensor_tensor ops with accum_out.
"""

import sys

if "/opt/trn_rl_repo" not in sys.path:
    sys.path.insert(0, "/opt/trn_rl_repo")

import numpy as np
from contextlib import ExitStack

import concourse.bass as bass  # noqa: F401
import concourse.tile as tile
from concourse import bacc, mybir
from concourse import masks

NCORES = 8
NPIX = 256
NVIS = 50000
NV_CORE = NVIS // NCORES            # 6250
NCHUNK = (NV_CORE + 127) // 128     # 49
NV_PAD = NCHUNK * 128               # 6272
BATCH = 512

CELL = np.float32(0.005) * np.float32(np.pi / 180.0 / 3600.0)
# conv is computed as (0.5*l + c + 0.5*r) per axis = 4x the Hann weights;
# fold the 1/4 together with the cell^2 pixel solid angle into one scalar.
SCALE = float(np.float32(np.float64(CELL) ** 2 / 4.0))
F32 = mybir.dt.float32
PI = float(np.pi)
MAGIC = float(np.float32(1.5 * 2 ** 23))  # round-to-nearest-int bias

_CACHE = {}


def _build():
    AF = mybir.ActivationFunctionType
    OP = mybir.AluOpType
    nc = bacc.Bacc("TRN2", target_bir_lowering=False, debug=False,
                   num_devices=NCORES)
    bc_ap = nc.dram_tensor("base_cube", [NPIX, NPIX], F32,
                           kind="ExternalInput").ap()
    uu_ap = nc.dram_tensor("uu", [1, NV_PAD], F32, kind="ExternalInput").ap()
    vv_ap = nc.dram_tensor("vv", [128, NCHUNK], F32,
                           kind="ExternalInput").ap()
    co_ap = nc.dram_tensor("coordsd", [1, NPIX], F32,
                           kind="ExternalInput").ap()
    out_ap = nc.dram_tensor("out_ri", [2, 128, NCHUNK], F32,
                            kind="ExternalOutput").ap()

    with tile.TileContext(nc) as tc, ExitStack() as ctx:
        persist = ctx.enter_context(tc.tile_pool(name="persist", bufs=1))
        imgT = [persist.tile([128, NPIX], F32, tag=f"imgT{i}",
                             name=f"imgT{i}") for i in range(2)]
        imgTn = [persist.tile([128, NPIX], F32, tag=f"imgTn{i}",
                              name=f"imgTn{i}") for i in range(2)]
        coords_b = persist.tile([128, NPIX], F32, tag="coords_b")
        coords_row = persist.tile([1, NPIX], F32, tag="coords_row")
        coords_neg = persist.tile([1, NPIX], F32, tag="coords_neg")
        quarter_row = persist.tile([1, 128], F32, tag="quarter_row")
        ones_kb = persist.tile([1, BATCH], F32, tag="ones_kb")
        stage = persist.tile([128, 2 * NCHUNK], F32, tag="stage")
        nc.vector.memset(quarter_row[:], 0.25)
        nc.vector.memset(ones_kb[:], 1.0)

        # ---------------- one-time image prep ----------------
        with tc.tile_pool(name="ssb", bufs=1) as ssb, \
             tc.tile_pool(name="sps", bufs=1, space="PSUM") as sps:
            ident = ssb.tile([128, 128], F32, tag="ident")
            masks.make_identity(nc, ident[:])
            ones_row = ssb.tile([1, 128], F32, tag="ones_row")
            nc.vector.memset(ones_row[:], 1.0)
            nc.sync.dma_start(coords_row[:], co_ap[:])
            nc.vector.tensor_scalar_mul(coords_neg[:], coords_row[:], -1.0)
            # coords broadcast across partitions (for the v-side tensor_scalar)
            ps_cb = sps.tile([128, NPIX], F32, tag="ps_cb")
            nc.tensor.matmul(ps_cb[:], ones_row[:], coords_row[:],
                             start=True, stop=True)
            nc.scalar.copy(coords_b[:], ps_cb[:])

            # softplus = Ln(1 + Exp(x)) into x-padded tiles
            impad = [ssb.tile([128, NPIX + 2], F32, tag=f"impad{i}",
                              name=f"impad{i}") for i in range(2)]
            for i in range(2):
                nc.vector.memset(impad[i][:], 0.0)
                raw = ssb.tile([128, NPIX], F32, tag=f"raw{i}")
                nc.sync.dma_start(raw[:], bc_ap[i * 128:(i + 1) * 128, :])
                expt = ssb.tile([128, NPIX], F32, tag=f"expt{i}",
                                name=f"expt{i}")
                nc.scalar.activation(expt[:], raw[:], AF.Exp)
                nc.scalar.activation(impad[i][:, 1:NPIX + 1], expt[:],
                                     AF.Ln, bias=1.0, scale=1.0)
            # conv along x: 0.5*(l+r) + c   (2x the Hann weights)
            cx = [ssb.tile([128, NPIX], F32, tag=f"cx{i}", name=f"cx{i}")
                  for i in range(2)]
            for i in range(2):
                t1 = ssb.tile([128, NPIX], F32, tag=f"t1_{i}")
                nc.vector.tensor_add(t1[:], impad[i][:, 0:NPIX],
                                     impad[i][:, 2:NPIX + 2])
                nc.vector.scalar_tensor_tensor(
                    cx[i][:], t1[:], 0.5, impad[i][:, 1:NPIX + 1],
                    op0=OP.mult, op1=OP.add)
            # transpose (y,x) -> (x,y) into y-padded tiles
            imp2 = [ssb.tile([128, NPIX + 2], F32, tag=f"imp2{i}",
                             name=f"imp2{i}") for i in range(2)]
            for xc in range(2):
                nc.vector.memset(imp2[xc][:], 0.0)
                for yc in range(2):
                    pst = sps.tile([128, 128], F32, tag=f"pst{xc}_{yc}")
                    nc.tensor.transpose(
                        pst[:], cx[yc][:, xc * 128:(xc + 1) * 128], ident[:])
                    nc.scalar.copy(
                        imp2[xc][:, 1 + yc * 128:1 + (yc + 1) * 128], pst[:])
            # conv along y + negated copy
            for xc in range(2):
                t2 = ssb.tile([128, NPIX], F32, tag=f"t2_{xc}")
                nc.vector.tensor_add(t2[:], imp2[xc][:, 0:NPIX],
                                     imp2[xc][:, 2:NPIX + 2])
                nc.vector.scalar_tensor_tensor(
                    imgT[xc][:], t2[:], 0.5, imp2[xc][:, 1:NPIX + 1],
                    op0=OP.mult, op1=OP.add)
                nc.vector.tensor_scalar_mul(imgTn[xc][:], imgT[xc][:], -1.0)

        # ---------------- main loop ----------------
        ups = ctx.enter_context(tc.tile_pool(name="ups", bufs=2, space="PSUM"))
        tps = ctx.enter_context(tc.tile_pool(name="tps", bufs=2, space="PSUM"))
        usb = ctx.enter_context(tc.tile_pool(name="usb", bufs=2))
        vsb = ctx.enter_context(tc.tile_pool(name="vsb", bufs=3))
        io = ctx.enter_context(tc.tile_pool(name="io", bufs=3))
        scr = ctx.enter_context(tc.tile_pool(name="scr", bufs=2))

        g = 0
        off = 0
        while off < NV_PAD:
            KB = min(BATCH, NV_PAD - off)
            nch = KB // 128
            c0 = off // 128
            u_row = io.tile([1, KB], F32, tag="u_row")
            nc.sync.dma_start(u_row[:], uu_ap[:, off:off + KB])
            v_col = io.tile([128, nch], F32, tag="v_col")
            nc.sync.dma_start(v_col[:], vv_ap[:, c0:c0 + nch])
            vn_col = io.tile([128, nch], F32, tag="vn_col")
            nc.vector.tensor_scalar_mul(vn_col[:], v_col[:], -1.0)

            # u-side phases in (x, k) layout: [a_im | a_re] per x-chunk
            phases = []
            for xc in range(2):
                # qab = [-q | q + 0.25], q[x, k] = coordsd[x] * uu[k]
                qab = ups.tile([128, 2 * KB], F32, tag="qab")
                nc.tensor.matmul(qab[:, 0:KB],
                                 coords_neg[:, xc * 128:(xc + 1) * 128],
                                 u_row[:], start=True, stop=True)
                nc.tensor.matmul(qab[:, KB:2 * KB],
                                 coords_row[:, xc * 128:(xc + 1) * 128],
                                 u_row[:], start=True, stop=False)
                nc.tensor.matmul(qab[:, KB:2 * KB], quarter_row[:],
                                 ones_kb[:, 0:KB], start=False, stop=True)
                # r = x - round(x), elementwise over both halves
                aa = usb.tile([128, 2 * KB], F32, tag="aa")
                nc.vector.tensor_scalar(aa[:], qab[:], MAGIC, MAGIC,
                                        op0=OP.add, op1=OP.subtract)
                vvu = usb.tile([128, 2 * KB], F32, tag="vvu")
                nc.vector.tensor_tensor(vvu[:], qab[:], aa[:],
                                        op=OP.subtract)
                ph = usb.tile([128, 2 * KB], F32, tag="ph")
                # sin(2*pi*r): halves become [-sin(2*pi*q) | cos(2*pi*q)]
                nc.scalar.activation(ph[:], vvu[:], AF.Sin,
                                     bias=0.0, scale=2.0 * PI)
                phases.append(ph)

            for c in range(nch):
                # v-side phases, (k, y) layout: [Cv | Svn]
                qq = vsb.tile([128, 2 * NPIX], F32, tag="qq")
                nc.vector.tensor_scalar(qq[:, 0:NPIX], coords_b[:],
                                        v_col[:, c:c + 1], 0.25,
                                        op0=OP.mult, op1=OP.add)
                nc.vector.tensor_scalar(qq[:, NPIX:2 * NPIX], coords_b[:],
                                        vn_col[:, c:c + 1], None,
                                        op0=OP.mult)
                aav = vsb.tile([128, 2 * NPIX], F32, tag="aav")
                nc.gpsimd.tensor_scalar(aav[:], qq[:], MAGIC, MAGIC,
                                        op0=OP.add, op1=OP.subtract)
                rrv = vsb.tile([128, 2 * NPIX], F32, tag="rrv")
                nc.gpsimd.tensor_tensor(rrv[:], qq[:], aav[:],
                                        op=OP.subtract)
                vph = vsb.tile([128, 2 * NPIX], F32, tag="vph")
                nc.scalar.activation(vph[:], rrv[:], AF.Sin,
                                     bias=0.0, scale=2.0 * PI)

                # T matmuls: [T_im | T_re | T_imneg] over x-chunks.
                # T_im (bank0) + T_imneg (bank1) groups interleave (distinct
                # PSUM banks, shared sin weights); T_re (bank0) starts only
                # after T_im's accumulation group has stopped.
                tcat = tps.tile([128, 3 * NPIX], F32, tag="tcat")
                for xc in range(2):
                    sl_sin = phases[xc][:, c * 128:(c + 1) * 128]
                    st, sp = (xc == 0), (xc == 1)
                    nc.tensor.matmul(tcat[:, 0:NPIX], sl_sin, imgT[xc][:],
                                     start=st, stop=sp)
                    nc.tensor.matmul(tcat[:, 2 * NPIX:3 * NPIX], sl_sin,
                                     imgTn[xc][:], start=st, stop=sp)
                for xc in range(2):
                    sl_cos = phases[xc][:, KB + c * 128:KB + (c + 1) * 128]
                    nc.tensor.matmul(tcat[:, NPIX:2 * NPIX], sl_cos,
                                     imgT[xc][:], start=(xc == 0),
                                     stop=(xc == 1))

                # fused combine + row-reduce:
                # im: sum(T_im*Cv + T_re*Svn), re: sum(T_re*Cv - T_im*Svn)
                dummy = scr.tile([128, 2 * NPIX], F32, tag="dummy")
                nc.vector.scalar_tensor_tensor(
                    dummy[:], tcat[:, 0:2 * NPIX], SCALE, vph[:],
                    op0=OP.mult, op1=OP.mult,
                    accum_out=stage[:, NCHUNK + g:NCHUNK + g + 1])
                dummy2 = scr.tile([128, 2 * NPIX], F32, tag="dummy")
                nc.vector.scalar_tensor_tensor(
                    dummy2[:], tcat[:, NPIX:3 * NPIX], SCALE, vph[:],
                    op0=OP.mult, op1=OP.mult,
                    accum_out=stage[:, g:g + 1])
                g += 1
            off += KB

        nc.sync.dma_start(out_ap[0], stage[:, 0:NCHUNK])
        nc.sync.dma_start(out_ap[1], stage[:, NCHUNK:2 * NCHUNK])

    nc.compile()
    return nc


class _Runner:
    """Persistent jitted 8-core SPMD executor (jit built once, reused)."""

    def __init__(self, nc):
        import jax
        from jax.sharding import Mesh, PartitionSpec
        from jax.experimental.shard_map import shard_map
        from concourse import bass2jax
        from concourse.bass2jax import install_neuronx_cc_hook

        install_neuronx_cc_hook()
        self.nc = nc
        partition_name = (nc.partition_id_tensor.name
                          if nc.partition_id_tensor else None)
        in_names, out_names, out_avals = [], [], []
        for alloc in nc.m.functions[0].allocations:
            if not isinstance(alloc, mybir.MemoryLocationSet):
                continue
            name = alloc.memorylocations[0].name
            if alloc.kind == "ExternalInput":
                if name != partition_name:
                    in_names.append(name)
            elif alloc.kind == "ExternalOutput":
                out_names.append(name)
                out_avals.append(jax.core.ShapedArray(
                    tuple(alloc.tensor_shape), mybir.dt.np(alloc.dtype)))
        self.in_names, self.out_names, self.out_avals = \
            in_names, out_names, out_avals
        n_params, n_outs = len(in_names), len(out_names)
        all_names = in_names + out_names
        if partition_name is not None:
            all_names = all_names + [partition_name]

        def _body(*args):
            operands = list(args)
            if partition_name is not None:
                operands.append(bass2jax.partition_id_tensor())
            outs = bass2jax._bass_exec_p.bind(
                *operands,
                out_avals=tuple(out_avals),
                in_names=tuple(all_names),
                out_names=tuple(out_names),
                lowering_input_output_aliases=(),
                sim_require_finite=True,
                sim_require_nnan=True,
                nc=nc,
            )
            return tuple(outs)

        devices = jax.devices()[:NCORES]
        mesh = Mesh(np.asarray(devices), ("core",))
        self._fn = jax.jit(
            shard_map(_body, mesh=mesh,
                      in_specs=(PartitionSpec("core"),) * (n_params + n_outs),
                      out_specs=(PartitionSpec("core"),) * n_outs,
                      check_rep=False),
            donate_argnums=tuple(range(n_params, n_params + n_outs)),
            keep_unused=True,
        )

    def __call__(self, in_maps):
        concat_in = [
            np.concatenate([np.asarray(m[name]) for m in in_maps], axis=0)
            for name in self.in_names
        ]
        zeros = [
            np.zeros((NCORES * a.shape[0], *a.shape[1:]), a.dtype)
            for a in self.out_avals
        ]
        outs = self._fn(*concat_in, *zeros)
        return [
            {name: np.asarray(outs[i]).reshape(NCORES, *self.out_avals[i].shape)[c]
             for i, name in enumerate(self.out_names)}
            for c in range(NCORES)
        ]


def _get_runner():
    if "runner" not in _CACHE:
        _CACHE["runner"] = _Runner(_build())
    return _CACHE["runner"]


def _coordsd():
    return ((np.arange(NPIX, dtype=np.float32) - np.float32(128.0))
            * CELL * np.float32(1000.0)).astype(np.float32)


def make_in_maps(base_cube, uu, vv):
    base = np.ascontiguousarray(np.asarray(base_cube)[0], dtype=np.float32)
    uu = np.asarray(uu, dtype=np.float32)
    vv = np.asarray(vv, dtype=np.float32)
    coordsd = _coordsd()[None, :]
    in_maps = []
    for c in range(NCORES):
        s = slice(c * NV_CORE, (c + 1) * NV_CORE)
        up = np.zeros(NV_PAD, np.float32)
        vp = np.zeros(NV_PAD, np.float32)
        up[:NV_CORE] = uu[s]
        vp[:NV_CORE] = vv[s]
        in_maps.append({
            "base_cube": base,
            "uu": up[None, :],
            "vv": np.ascontiguousarray(vp.reshape(NCHUNK, 128).T),
            "coordsd": coordsd,
        })
    return in_maps


def assemble(results):
    out = np.empty((1, NVIS), np.complex64)
    for c in range(NCORES):
        ri = results[c]["out_ri"]  # (2, 128, NCHUNK)
        vis = (ri[0] + 1j * ri[1]).astype(np.complex64)
        flat = vis.T.reshape(-1)   # k = chunk*128 + partition
        out[0, c * NV_CORE:(c + 1) * NV_CORE] = flat[:NV_CORE]
    return out


def kernel(base_cube, uu, vv):
    runner = _get_runner()
    return assemble(runner(make_in_maps(base_cube, uu, vv)))

